# revision 34
# baseline (speedup 1.0000x reference)
"""Trainium2 Bass kernel for single-head attention.

Reference computation (per batch b):
    q = x @ Wq; k = x @ Wk; v = x @ Wv            # [T, D]
    S = (q @ k.T) * C**-0.5                        # [T, T]
    P = softmax(S, axis=-1)
    out = P @ v                                    # [T, D]

Shapes: x [16, 4096, 1024] f32, W* [1024, 64] f32, out [16, 4096, 64] f32.

Sharding: data-parallel over batch across 8 cores (2 batches/core), weights
replicated.

Per-core kernel strategy (all on-chip compute in bf16, fp32 accumulation):
 - x is cast to bf16 on host; loaded transposed (xT, C on partitions) via
   DMA-transpose so the contraction dim of the projections is on partitions.
 - QK projection fused: lhsT = [Wq | Wk] -> qkT [128, T] (qT rows 0:64,
   kT rows 64:128); an SBUF->SBUF DMA builds the swapped copy [kT; qT] so
   score matmuls for two k-tiles run CONCURRENTLY on PE row groups 0-63 and
   64-127 (matmul operands must live in the issuing row group's partitions).
 - V projected to vT [64, T], then PE-transposed to natural v tiles
   [128, 65] with a ones column appended (softmax denominator comes out of
   the PV matmul for free).
 - Scores computed transposed: S^T tile [128 k, 512 q] = kT_tile.T @ qT.
   exp(S^T / 32) runs on the ACT engine straight out of PSUM into bf16
   SBUF (scale folded into the activation's free affine), [128, 1024] per
   op (two k-tiles) to amortize the ~400ns fixed cost. ACT is the
   bottleneck engine (~330us busy per core).
 - PV: acc[65, 512] += [v|1]_tile.T @ P^T accumulated over all 32 k-tiles in
   PSUM. Row 64 of acc is the softmax denominator.
 - Finalize: acc -> SBUF, PE-transpose per 128-q tile, multiply by the
   reciprocal of the denominator (per-partition scalar), DMA out fp32.

Engine queues are FIFO, so emission order = per-engine execution order. The
emission is software-pipelined: each batch's projection work is emitted in
single-matmul units interleaved through the previous batch's attention
stream (filling PE slack under the ACT-paced exp pipeline), and each PV
pair is emitted one step behind its exp so the PE queue never head-of-line
blocks waiting for the activation engine.
"""

import numpy as np
import ml_dtypes

B, T, C, D = 16, 4096, 1024, 64
N_CORES = 8
NB = B // N_CORES  # batches per core
P = 128
KT = T // P  # 32 k-tiles per batch
CT = C // P  # 8 contraction tiles for projections
NQ = 1024  # q-chunk processed per exp/PV step
NQC = T // NQ  # 4 q-chunks per batch
T2 = T // 2  # half-batch T extent for xT staging
SCALE = float(C) ** -0.5

# Wq/Wk are pre-scaled by sqrt(SCALE) on the host, so the score matmuls
# produce x = s * SCALE directly in PSUM (|x| <~ 2.1; the softmax argument).
#
# exp(x) is split between the ACT engine (true exp) and the DVE (a custom
# 8-stage op computing ((x+A)^2+B)*((x+C)^2+Dc))^2 ~= K*exp(x) over
# [-2.4, 2.4], max rel err ~0.25%). The ACT tiles are biased by ln(K) so
# both engines produce K*exp(x); the common factor K cancels in softmax.
EXP_A = 3.90057152
EXP_B = 2.61858617
EXP_C = 0.69857781
EXP_D = 22.35692466
EXP_LNK = 12.021637894239786  # ln(((A^2+B)(C^2+D))^2) centering constant
# Custom DVE instructions need mybir.codegen_inst_isa_subclasses() before
# serialization (walrus rejects the zero-length InstISA otherwise).
USE_DVE_EXP = True
# which of the 16 (nq, k-pair) steps run exp on DVE (rest on ACT): 8/16,
# interleaved so both engines stream concurrently. The PSUM->SBUF copies
# ride the ACT engine (gpsimd has no PSUM port), so ACT gets fewer exps.
DVE_TP = frozenset({1, 3, 5, 7, 9, 11, 13, 15})

_BF16 = ml_dtypes.bfloat16

_cached_nc = None


def _register_exp_op():
    """Register the custom DVE exp-approx op (idempotent). Must happen in
    the same process that traces/compiles the kernel."""
    from concourse import dve_ops
    from concourse.dve_spec import C0, C1, C2, Spec, Src0, Src1, sq

    for op in dve_ops.OPS:
        if op.name == "EXP4SQ_ANT":
            return op

    def _ref(in0, in1, s0, s1, imm2):
        x = in0.astype(np.float32)
        q = ((x + s0) ** 2 + s1) * ((x + imm2) ** 2 + in1)
        return (q * q).astype(np.float32)

    op = dve_ops.DveOp(
        "EXP4SQ_ANT",
        Spec(
            body=sq((sq(Src0 + C0) + C1) * (sq(Src0 + C2) + Src1)),
            reference=_ref,
        ),
        subdim=False,
        uops_sha={"v3": "f9fe5afb45e9fdf9"},
    )
    dve_ops.OPS.append(op)
    dve_ops.CUSTOM_DVE_SPECS[op.name] = op.spec
    dve_ops._SUB_OPCODE_FOR_NAME[op.name] = (
        dve_ops._CUSTOM_DVE_ROW_BASE + len(dve_ops.OPS) - 1
    )
    return op


def _patch_tile_drain():
    """walrus in this toolchain rejects instructions with >1 sync wait on the
    Drain opcode; split the TileContext exit drain into 1-wait drains."""
    import bass_rust
    import concourse.tile as tile
    from concourse.tile import ScopedClock

    if getattr(tile.TileContext, "_drain_split_patched", False):
        return

    def _split_drain_and_barrier(self, tick_clock, wait_clock):
        drain_inst = self.nc.sync.drain()
        wait_clock.add_sem_waits(
            drain_inst.ins, ScopedClock({None: tick_clock.global_clock})
        )
        si = drain_inst.ins.sync_info
        waits = list(si.on_wait) if si is not None else []
        if len(waits) > 1:
            si.on_wait = waits[:1]
            drain_inst.ins.sync_info = si
            for i in range(1, len(waits)):
                extra = self.nc.sync.drain()
                extra.ins.sync_info = bass_rust.SyncInfo(
                    on_wait=waits[i : i + 1], on_update=[]
                )
        self.nc.all_engine_barrier()
        popped = self.nc._tile_sem_poison_stack.pop()
        assert popped is self._sem_poison
        self.nc.clear_and_free_semaphores(list(self.sems.allocated().values()))
        self.nc.all_engine_barrier()

    tile.TileContext._drain_and_barrier = _split_drain_and_barrier
    tile.TileContext._drain_split_patched = True


def _split_multi_wait_instructions(nc):
    """walrus in this toolchain allows at most one sync wait per instruction.
    Hoist extra waits onto nop instructions inserted immediately before, on
    the same engine (engine queues are FIFO, so ordering is preserved)."""
    import bass_rust

    for func in nc.m.functions:
        for bb in func.blocks:
            insts = list(bb.instructions)
            if not any(
                ins.sync_info is not None and len(ins.sync_info.on_wait) > 1
                for ins in insts
            ):
                continue
            cur_bb = nc.cur_bb.bb if nc.cur_bb is not None else None
            cur_snapshot = list(cur_bb.instructions) if cur_bb is not None else None
            new_list = []
            for ins in insts:
                si = ins.sync_info
                if si is not None and len(si.on_wait) > 1:
                    waits = list(si.on_wait)
                    eng = nc.engines[ins.engine]
                    for w in waits[:-1]:
                        nop = eng.nop(nofuse=True, hint="wait_split")
                        nop.ins.sync_info = bass_rust.SyncInfo(
                            on_wait=[w], on_update=[]
                        )
                        new_list.append(nop.ins)
                    si.on_wait = waits[-1:]
                    ins.sync_info = si
                new_list.append(ins)
            if cur_bb is not None and cur_bb.name != bb.name:
                # nops were appended to cur_bb; strip them from there
                cur_bb.instructions = cur_snapshot
            bb.instructions = new_list


def build_nc(repeat=1, loop_n=0, packed_s=True, ablate=None, dma_vnat=False):
    """Build the per-core Bass program (identical on all 8 cores).

    repeat > 1 re-runs the whole per-core workload that many times (writing
    the same outputs); loop_n > 0 additionally wraps the workload in an
    on-device For_i loop with that many iterations. Both are used by the
    timing harness to separate HW execution time from the (large, noisy)
    dispatch overhead of this environment via the wall-time slope."""
    import concourse.bass as bass
    import concourse.tile as tile
    from concourse import mybir
    from concourse.bass import ds, ts
    from concourse.masks import make_identity

    _patch_tile_drain()
    exp_op = _register_exp_op()

    f32 = mybir.dt.float32
    bf16 = mybir.dt.bfloat16

    nc = bass.Bass()
    x_in = nc.dram_tensor("x", [NB, C, T], bf16, kind="ExternalInput")
    wqk_in = nc.dram_tensor("wqk", [C, 2 * D], bf16, kind="ExternalInput")
    wv_in = nc.dram_tensor("wv", [C, D], bf16, kind="ExternalInput")
    out_dram = nc.dram_tensor("out", [NB, T, D], f32, kind="ExternalOutput")

    with tile.TileContext(nc) as tc:
        with (
            tc.tile_pool(name="weights", bufs=1) as wpool,
            tc.tile_pool(name="xT", bufs=2) as xpool,
            tc.tile_pool(name="qk", bufs=2) as qkpool,
            tc.tile_pool(name="kT", bufs=2) as ktpool,
            tc.tile_pool(name="vT", bufs=2) as vtpool,
            tc.tile_pool(name="vs", bufs=2) as vspool,
            tc.tile_pool(name="pt", bufs=6) as ptpool,
            tc.tile_pool(name="oT", bufs=2) as otpool,
            tc.tile_pool(name="r", bufs=4) as rpool,
            tc.tile_pool(name="osb", bufs=3) as opool,
            tc.tile_pool(name="psbig", bufs=5, space="PSUM") as psbig,
            tc.tile_pool(name="psacc", bufs=2, space="PSUM") as psacc,
            tc.tile_pool(name="psout", bufs=1, space="PSUM") as psout,
        ):
            # --- constants ---
            wqk_sb = wpool.tile([P, CT, 2 * D], bf16)
            nc.sync.dma_start(
                wqk_sb[:], wqk_in.rearrange("(c p) m -> p c m", p=P)
            )
            wv_sb = wpool.tile([P, CT, D], bf16)
            nc.sync.dma_start(wv_sb[:], wv_in.rearrange("(c p) m -> p c m", p=P))
            ident_bf = wpool.tile([P, P], bf16)
            make_identity(nc, ident_bf[:])
            ident_f32 = wpool.tile([P, P], f32)
            make_identity(nc, ident_f32[:])
            if ablate == "noact":
                pt_const = wpool.tile([P, 1024], bf16)
                nc.gpsimd.memset(pt_const[:], 0.0078)
            sink = wpool.tile([P, 1], f32)
            nc.gpsimd.memset(sink[:], 0.0)
            # per-partition scalar carrying the 4th poly coefficient for the
            # custom DVE exp op (TTSS shape: src1 must be [P, 1])
            dconst = wpool.tile([P, 1], f32)
            nc.gpsimd.memset(dconst[:], EXP_D)
            lnk_bias = wpool.tile([P, 1], f32)
            nc.gpsimd.memset(lnk_bias[:], EXP_LNK)

            def _copy_eng(out, in_):
                # PSUM->SBUF copies: on ACT when the DVE carries half the
                # exp load, on DVE otherwise
                if USE_DVE_EXP:
                    nc.scalar.copy(out, in_)
                else:
                    nc.vector.tensor_copy(out, in_)

            import contextlib

            # Per-batch state (tiles shared between proj- and attn-units)
            state = {}

            def proj_units(b):
                """Projection pipeline for batch b as a list of emission
                units. Interleaved into the previous batch's attention
                stream so the (FIFO) PE queue has projection work filling
                the slack of the ACT-paced attention groups."""
                units = []

                def load(h, ch, b=b):
                    # one 512-token chunk of one half-batch, all c-tiles, in
                    # consumption order so projections can start after the
                    # first chunk lands. x is pre-transposed on the host to
                    # [C, T], so this is a plain strided DMA (no XBAR).
                    if (b, "xh", h) not in state:
                        state[(b, "xh", h)] = xpool.tile(
                            [P, CT, T2], bf16, name=f"xh_{b}_{h}", tag="xh"
                        )
                    xt = state[(b, "xh", h)]
                    t0 = h * T2 + ch * 512
                    nc.sync.dma_start(
                        out=xt[:, :, ds(ch * 512, 512)],
                        in_=x_in[b].rearrange("(c p) t -> p c t", p=P)[
                            :, :, ds(t0, 512)
                        ],
                    )

                def qk_mm(n, c, b=b):
                    if (b, "qkT") not in state:
                        state[(b, "qkT")] = qkpool.tile(
                            [P, T], bf16, name=f"qkT_{b}", tag="qkT"
                        )
                    if c == 0:
                        state[(b, "pps")] = psbig.tile(
                            [P, 512], f32, name=f"pps_{b}_{n}", tag="big"
                        )
                    ps = state[(b, "pps")]
                    xt = state[(b, "xh", n // (T2 // 512))]
                    off = (n % (T2 // 512)) * 512
                    nc.tensor.matmul(
                        ps[:],
                        wqk_sb[:, c, :],
                        xt[:, c, ds(off, 512)],
                        start=(c == 0),
                        stop=(c == CT - 1),
                    )
                    if c == CT - 1:
                        _copy_eng(
                            state[(b, "qkT")][:, ts(n, 512)], ps[:]
                        )

                def qk_swap(n, b=b):
                    # swapped copy [kT; qT] so both PE row groups can run
                    # score matmuls concurrently (operands must live in the
                    # issuing row group's partition range); per 512-chunk so
                    # early chunks are usable before the projection finishes
                    if (b, "qkT2") not in state:
                        state[(b, "qkT2")] = ktpool.tile(
                            [P, T], bf16, name=f"qkT2_{b}", tag="qkT2"
                        )
                    qkT = state[(b, "qkT")]
                    qkT2 = state[(b, "qkT2")]
                    nc.sync.dma_start(
                        qkT2[0:64, ts(n, 512)], qkT[64:128, ts(n, 512)]
                    )
                    nc.sync.dma_start(
                        qkT2[64:128, ts(n, 512)], qkT[0:64, ts(n, 512)]
                    )

                def v_mm(n, c, b=b):
                    if (b, "vT") not in state:
                        state[(b, "vT")] = vtpool.tile(
                            [64, T], bf16, name=f"vT_{b}", tag="vT"
                        )
                    if c == 0:
                        state[(b, "vps")] = psbig.tile(
                            [64, 512], f32, name=f"vps_{b}_{n}", tag="big"
                        )
                    ps = state[(b, "vps")]
                    xt = state[(b, "xh", n // (T2 // 512))]
                    off = (n % (T2 // 512)) * 512
                    nc.tensor.matmul(
                        ps[:],
                        wv_sb[:, c, :],
                        xt[:, c, ds(off, 512)],
                        start=(c == 0),
                        stop=(c == CT - 1),
                    )
                    if c == CT - 1:
                        _copy_eng(
                            state[(b, "vT")][:, ts(n, 512)], ps[:]
                        )



                def v_nat(g, b=b):
                    # transpose 2 k-tiles of vT into natural layout via
                    # DMA-transpose (keeps the PE free; bf16 is supported)
                    if (b, "vsb") not in state:
                        vsb = vspool.tile([P, KT, D + 1], bf16, name=f"vsb_{b}", tag="vsb")
                        nc.gpsimd.memset(vsb[:, :, D], 1.0)
                        state[(b, "vsb")] = vsb
                    vsb = state[(b, "vsb")]
                    vT = state[(b, "vT")]
                    for t in range(g * 2, (g + 1) * 2):
                        if dma_vnat:
                            nc.sync.dma_start(
                                out=vsb[:, t, 0:D],
                                in_=vT[:, ts(t, P)],
                                transpose=True,
                            )
                        else:
                            pst = psout.tile([P, D], bf16, tag="ot")
                            nc.tensor.transpose(
                                pst[:], vT[:, ts(t, P)], ident_bf[0:64, 0:64]
                            )
                            nc.vector.tensor_copy(vsb[:, t, 0:D], pst[:])

                # per-512-chunk blocks in consumption order: x-load, QK
                # projection + copy, swap, V projection + copy, v-natural
                for n in range(T // 512):
                    units.append(lambda n=n: load(n // 4, n % 4))
                    for c in range(CT):
                        units.append(lambda n=n, c=c: qk_mm(n, c))
                    units.append(lambda n=n: qk_swap(n))
                    for c in range(CT):
                        units.append(lambda n=n, c=c: v_mm(n, c))
                    units.append(lambda n=n: v_nat(2 * n))
                    units.append(lambda n=n: v_nat(2 * n + 1))
                return units

            def attn_units(b):
                """Attention + finalize for batch b as emission units:
                one unit per (nq, k-tile-pair) plus one finalize unit per
                nq chunk."""
                units = []

                def emit_pv(nq, kt, pt_ap, b=b):
                    acc = state[(b, "acc", nq)]
                    vsb = state[(b, "vsb")]
                    nc.tensor.matmul(
                        acc[:],
                        vsb[:, kt, :],
                        pt_ap,
                        start=(kt == 0),
                        stop=(kt == KT - 1),
                    )

                def flush_pv(b, n=None):
                    pend = state.setdefault((b, "pv"), [])
                    k = len(pend) if n is None else n
                    for _ in range(k):
                        emit_pv(*pend.pop(0))

                def attn_step(nq, tp, b=b):
                    # one k-tile pair: two S matmuls into one [128,1024]
                    # PSUM tile, pending-PV drains between them, one exp
                    # over the pair (ACT, or DVE for odd pairs when the
                    # custom DVE op is enabled).
                    if (b, "oT") not in state:
                        state[(b, "oT")] = otpool.tile([65, T], f32, name=f"oT_{b}", tag="oT")
                    if tp == 0:
                        state[(b, "acc", nq)] = psacc.tile(
                            [65, 512], f32, name=f"acc_{b}_{nq}", tag="acc"
                        )
                    qkT = state[(b, "qkT")]
                    qkT2 = state[(b, "qkT2")]
                    s = psbig.tile([P, 1024], f32, tag="big")
                    nc.tensor.matmul(
                        s[:, 0:512],
                        qkT2[0:64, ts(2 * tp, P)],
                        qkT[0:64, ds(nq * 512, 512)],
                        start=True,
                        stop=True,
                    )
                    pend = state.setdefault((b, "pv"), [])
                    if len(pend) > 1:
                        flush_pv(b, 1)
                    nc.tensor.matmul(
                        s[:, 512:1024],
                        qkT[64:128, ts(2 * tp + 1, P)],
                        qkT2[64:128, ds(nq * 512, 512)],
                        start=True,
                        stop=True,
                    )
                    if len(pend) > 0:
                        flush_pv(b, 1)
                    if ablate == "noact":
                        nc.vector.tensor_copy(sink[0:1, 0:1], s[0:1, 0:1])
                        pt0 = pt_const
                        pt1 = pt_const
                    elif USE_DVE_EXP and tp % 2 == 1:
                        pt0 = ptpool.tile([P, 1024], bf16)
                        nc.vector._custom_dve(
                            exp_op,
                            out=pt0[:],
                            in0=s[:],
                            in1=dconst[:],
                            s0=EXP_A,
                            s1=EXP_B,
                            imm2=EXP_C,
                        )
                        pt1 = None
                    else:
                        pt0 = ptpool.tile([P, 1024], bf16)
                        nc.scalar.activation(
                            pt0[:],
                            s[:],
                            mybir.ActivationFunctionType.Exp,
                            scale=1.0,
                            bias=lnk_bias[:],
                        )
                        pt1 = None
                    if ablate == "nopv":
                        nc.vector.tensor_copy(sink[0:1, 0:1], pt0[0:1, 0:1])
                        return
                    pend.append((nq, 2 * tp, pt0[:, 0:512]))
                    pend.append((nq, 2 * tp + 1, pt0[:, 512:1024]))

                def finalize(nq, b=b):
                    if ablate == "nopv":
                        return
                    flush_pv(b)
                    oT = state[(b, "oT")]
                    acc = state.pop((b, "acc", nq))
                    _copy_eng(oT[:, ds(nq * 512, 512)], acc[:])
                    osb = opool.tile([P, 4, D], f32)
                    for j in range(4):
                        m = nq * 4 + j
                        pso = psout.tile([P, 65], f32, tag="ot")
                        nc.tensor.transpose(
                            pso[:], oT[:, ds(m * P, P)], ident_f32[0:65, 0:65]
                        )
                        r = rpool.tile([P, 1], f32)
                        nc.vector.reciprocal(r[:], pso[:, 64:65])
                        nc.vector.tensor_scalar_mul(
                            osb[:, j, :], pso[:, 0:D], r[:]
                        )
                    nc.sync.dma_start(
                        out=out_dram[b].rearrange("(n p) d -> p n d", p=P)[
                            :, ds(nq * 4, 4), :
                        ],
                        in_=osb[:],
                    )

                pending_fin = []
                for nq in range(T // 512):
                    for tp in range(KT // 2):
                        units.append(lambda nq=nq, tp=tp: attn_step(nq, tp))
                        if tp == 1 and pending_fin:
                            units.append(pending_fin.pop())
                    pending_fin.append(lambda nq=nq: finalize(nq))
                units.extend(reversed(pending_fin))
                return units

            def emit_interleaved(attn, proj):
                """Emit attention units with projection units of the next
                batch spread evenly through them (PE queues are FIFO, so
                emission order is execution order per engine)."""
                if not proj:
                    for u in attn:
                        u()
                    return
                ratio = len(attn) / len(proj)
                pi = 0
                for i, u in enumerate(attn):
                    u()
                    while pi < len(proj) and (pi + 1) * ratio <= i + 1:
                        proj[pi]()
                        pi += 1
                while pi < len(proj):
                    proj[pi]()
                    pi += 1

            loop_ctx = (
                tc.For_i(0, loop_n, 1) if loop_n else contextlib.nullcontext()
            )
            with loop_ctx:
                batches = [b for _ in range(repeat) for b in range(NB)]
                # warm start: weave the first batch's nq=0 attention steps
                # into its own (DMA-bound) projection stream as soon as the
                # per-chunk dependencies allow
                proj0 = proj_units(batches[0])
                attn0 = attn_units(batches[0])
                BL = 20  # units per 512-chunk projection block
                assert len(proj0) == 8 * BL
                for u in proj0[0:BL]:
                    u()
                ai = 0
                for j in range(1, 8):
                    for u in proj0[BL * j : BL * (j + 1)]:
                        u()
                    hi = 16 if j == 7 else 2 * j
                    while ai < hi:
                        attn0[ai]()
                        ai += 1
                for i, b in enumerate(batches):
                    cur = attn0[ai:] if i == 0 else attn_units(b)
                    nxt = (
                        proj_units(batches[i + 1])
                        if i + 1 < len(batches)
                        else []
                    )
                    emit_interleaved(cur, nxt)

            if ablate == "nopv":
                dummy = wpool.tile([P, D], f32)
                nc.gpsimd.memset(dummy[:], 0.0)
                for b_ in range(NB):
                    nc.sync.dma_start(
                        out_dram[b_, 0:P, :], dummy[:]
                    )

    _split_multi_wait_instructions(nc)
    return nc


def _get_nc():
    global _cached_nc
    if _cached_nc is None:
        _cached_nc = build_nc()
    return _cached_nc


def make_in_maps(x, Wq, Wk, Wv):
    """Host-side prep: cast to bf16, fuse Wq|Wk (pre-scaled by sqrt(SCALE)
    so scores land in PSUM already multiplied by SCALE), shard batch."""
    # x pre-transposed to [B, C, T] so the device x loads are plain DMAs
    xb = np.ascontiguousarray(
        np.asarray(x).astype(_BF16).transpose(0, 2, 1)
    )
    rs = np.float32(np.sqrt(SCALE))
    wqk = (np.concatenate([Wq, Wk], axis=1) * rs).astype(_BF16)
    wv = np.ascontiguousarray(Wv).astype(_BF16)
    return [
        {"x": xb[i * NB : (i + 1) * NB], "wqk": wqk, "wv": wv}
        for i in range(N_CORES)
    ]


def kernel(x, Wq, Wk, Wv):
    from concourse.bass_utils import run_bass_kernel_spmd

    nc = _get_nc()
    in_maps = make_in_maps(x, Wq, Wk, Wv)
    res = run_bass_kernel_spmd(nc, in_maps, list(range(N_CORES)))
    return np.concatenate(
        [res.results[i]["out"] for i in range(N_CORES)], axis=0
    ).astype(np.float32)

